# revision 50
# baseline (speedup 1.0000x reference)
"""MHSA3D Trainium2 kernel v3: 8-way head-parallel attention with the
exp wall split across ScalarE + VectorE (single-op int16 Schraudolph),
host-side softmax normalization, and a fully continuous qk->exp->PV
pipeline.  ~124us vs the 163us v2 baseline.

Problem (hardcoded): B=1, C=128, D=H=W=16 -> N=4096 tokens, 8 heads,
dh=16, dv=128.  One head per NeuronCore.

Key structure per core (what profiling showed and what each piece is
for):
- The wall is exp over N^2=16.7M logits.  ScalarE ACT does ~60% of the
  [128,1024] tiles at ~1147ns; VectorE does ~40% via a ONE-op
  Schraudolph: tensor_scalar(mult 2^7, add 128*(127-0.058)) converting
  f32 PSUM logits straight to int16 whose bits ARE the bf16 exp value
  (pt tile written through a .bitcast(i16) view; ~1223ns, ~2% std err
  that washes out in softmax).  DVE_NUM tunes the split.
- The PSUM ring of three [128,1024] logit tiles is the pace-setter:
  slot cycle = exp (1.15-1.22us) + qk refill (0.63us cold) + ~0.3us of
  semaphore hops, i.e. ~0.71us/tile floor at 3 slots.  PSUM is full
  (6 ring banks + PV accumulator + staging), and the PE array never
  un-throttles (HAM needs ~3.4us of CONTINUOUS busy; this kernel's PE
  duty is bursty), so qk matmuls run at 1.2GHz.
- qk uses M=128 tiles: per half-batch, 2 matmuls at row groups 2h,2h+1
  (tile_position (32r,0)), sharing full-width fp16 weight loads (FWL).
  q'/k' replicated across the 4 PE row groups by zero-padded projection
  weights.
- PV is 4-way col-tiled (tile_position (0,32s)) with a leading
  ones-column in vaug producing the softmax denominator rows; rounds
  (4 MMs, one j-chunk) are popped one per 2 tiles so their all-row PE
  occupancy interleaves with qk instead of forming 2.4us bubbles.
  K-splitting PV across row positions to overlap qk is NOT possible:
  PSUM accumulation from two different tile_position rows faults on
  TRN2 (verified by minimal repro).
- Normalization happens on host: o_d carries [denom; 16 v-rows] per
  strip (bf16), kernel() divides in numpy.  This removed the whole
  reciprocal/broadcast/multiply tail (~11us).
- Startup (DMA completion latency is ~5us per transfer and stacks FIFO
  per ring): a single "head" transfer carries x[:,0:2048] + wq4 + wk4 +
  bq4 so the first projections/qk need exactly one sync-ring
  completion; bk4 chunk 0 rides the scalar HWDGE ring; bk4's row-group
  replication happens on-device via SBUF->SBUF DMA (576KB less HBM).
  dram rows must be 8-byte aligned or the DMA corrupts (hence the
  padded head width).
- The main loop is one continuous stream over all 128 tiles: exp(T-3)
  is emitted with qk(T) so no leftover-exp clump forms at j-batch or
  q-half boundaries; projections for later eighths ride the DMA-bound
  first block; the v projection is staged through `spare` in 8 groups
  interleaved with j-batches.
"""

import numpy as np

NHEADS = 8
DV = 128
DH = DV // NHEADS  # 16
C = 128
N = 4096
IW = 512           # i-eighth width
NE = N // IW       # 8
NCHUNK = 32        # 128-wide j-chunks
LOG2E = 1.4426950408889634
LN2 = 0.6931471805599453
MAGIC16 = 128.0 * (127.0 - 0.058)
SCALE16 = 128.0

# j-batches per eighth: 8x 512, each written as two 2-bank ring tiles
JB = [512 * k for k in range(8)]

# exp tiles assigned to the DVE (Schraudolph): DVE_NUM out of every
# DVE_DEN tiles, bresenham-spread.
DVE_NUM = 13
DVE_DEN = 32

_compiled = None


def _build_program():
    import concourse.bacc as bacc
    import concourse.mybir as mybir
    import concourse.tile as tile

    f32 = mybir.dt.float32
    bf16 = mybir.dt.bfloat16
    fp16 = mybir.dt.float16
    i16 = mybir.dt.int16
    EXP = mybir.ActivationFunctionType.Exp
    IDENT = mybir.ActivationFunctionType.Identity
    ADD = mybir.AluOpType.add
    MULT = mybir.AluOpType.mult

    nc = bacc.Bacc("TRN2", target_bir_lowering=False, debug=False,
                   num_devices=NHEADS)

    # head packs every byte the first projections need -- x[:, 0:1024],
    # wq4, wk4, bq4(f32 as 2 fp16 cols) -- into ONE transfer, because
    # per-ring DMA completion latency (~5us) stacks FIFO per ring.
    head_d = nc.dram_tensor("head", [C, 2432], fp16, kind="ExternalInput")
    x_d = nc.dram_tensor("x", [C, N], fp16, kind="ExternalInput")
    wv_d = nc.dram_tensor("wv", [C, DH], fp16, kind="ExternalInput")
    # k bias+emb: chunk 0 pre-replicated across the 4 row groups; chunks
    # 1-3 as a single row group, replicated on-device via SBUF-to-SBUF
    # DMA (saves 576KB of HBM traffic during the startup crunch)
    bk4a_d = nc.dram_tensor("bk4a", [128, 1024], bf16, kind="ExternalInput")
    bk4b_d = nc.dram_tensor("bk4b", [32, 3072], bf16, kind="ExternalInput")
    bvp_d = nc.dram_tensor("bvp", [128, 512], bf16, kind="ExternalInput")
    o_d = nc.dram_tensor("out", [128, 1024], bf16, kind="ExternalOutput")

    with tile.TileContext(nc) as tc:
        with (
            tc.tile_pool(name="const", bufs=1) as const,
            tc.tile_pool(name="pt", bufs=24) as ptp,
            tc.tile_pool(name="os", bufs=2) as osp,
            tc.tile_pool(name="ps", bufs=1, space="PSUM") as psp,
        ):
            head_s = const.tile([C, 2432], fp16)
            x_s = const.tile([C, N], fp16)
            wv_s = const.tile([C, DH], fp16)
            bk4_s = const.tile([128, N], bf16)
            bvp_s = const.tile([128, 512], bf16)
            qzc = [const.tile([128, 512], fp16, name=f"qzc{i}")
                   for i in range(8)]
            kzc = [const.tile([128, 512], fp16, name=f"kzc{i}")
                   for i in range(8)]
            vaugT = const.tile([128, 32 * NCHUNK], bf16)
            zerob = const.tile([128, 1], f32)
            scr1 = const.tile([128, 1], f32)

            st3 = [psp.tile([128, 1024], f32, name=f"st{i}")
                   for i in range(3)]
            accb = psp.tile([128, 512], f32)
            spare = psp.tile([128, 512], f32)

            # ---- input DMAs -----------------------------------------
            # critical-path DMAs ride the two HWDGE rings (sync +
            # scalar, ~600ns first byte); bulk/late ones go SWDGE
            # (gpsimd, ~2us fixed cost each).
            nc.gpsimd.memset(zerob[:], 0.0)
            # one critical first-transfer per ring: head on sync, bk4
            # chunk 0 on scalar, bk4 tail + replicas on gpsimd SWDGE.
            nc.sync.dma_start(head_s[:], head_d.ap())
            nc.scalar.dma_start(bk4_s[:, 0:1024], bk4a_d.ap())
            nc.scalar.activation(scr1[:], zerob[:], EXP, bias=zerob[:])
            nc.gpsimd.dma_start(bk4_s[0:32, 1024:4096], bk4b_d.ap())
            nc.sync.dma_start(x_s[:, 2048:4096], x_d.ap()[:, 2048:4096])
            for r in range(1, 4):
                nc.gpsimd.dma_start(bk4_s[32 * r:32 * r + 32, 1024:4096],
                                    bk4_s[0:32, 1024:4096])
            nc.gpsimd.dma_start(wv_s[:], wv_d.ap())
            nc.gpsimd.dma_start(bvp_s[:], bvp_d.ap())
            wq4_s = head_s[:, 2048:2176]
            wk4_s = head_s[:, 2176:2304]
            bq4_s = head_s[:, 2304:2306].bitcast(f32)
            nc.vector.memset(vaugT[:], 0.0)
            # ones column of vaug (denominator rows), whole tile upfront
            va3 = vaugT[:].rearrange("p (c s) -> p c s", s=32)
            nc.vector.memset(va3[:, :, 0:1], 1.0)

            # ---- projection emitters ------------------------------
            def xsrc(lo, hi):
                return head_s[:, lo:hi] if hi <= 2048 else x_s[:, lo:hi]

            def emit_kproj(i, sl):
                nc.tensor.matmul(sl, lhsT=wk4_s,
                                 rhs=xsrc(i * 512, (i + 1) * 512),
                                 start=True, stop=True)
                nc.vector.tensor_tensor(kzc[i][:], sl,
                                        bk4_s[:, i * 512:(i + 1) * 512], ADD)

            def emit_qproj(i, sl):
                nc.tensor.matmul(sl, lhsT=wq4_s,
                                 rhs=xsrc(i * 512, (i + 1) * 512),
                                 start=True, stop=True)
                nc.scalar.activation(qzc[i][:], sl, IDENT, bias=bq4_s)

            def emit_vgroup(g):
                # v^T chunks 4g..4g+3 staged through spare, then folded
                # with bv into vaug chunks 4g..4g+3
                for m in range(4 * g, 4 * g + 4):
                    nc.tensor.matmul(
                        spare[:, 16 * (m - 4 * g) + 64 * g:
                              16 * (m - 4 * g) + 64 * g + 16],
                        lhsT=xsrc(128 * m, 128 * (m + 1)),
                        rhs=wv_s[:], start=True, stop=True)
                vp3 = spare[:, 64 * g:64 * g + 64].rearrange(
                    "p (c s) -> p c s", s=DH)
                bv3 = bvp_s[:, 64 * g:64 * g + 64].rearrange(
                    "p (c s) -> p c s", s=DH)
                nc.vector.tensor_tensor(
                    va3[:, 4 * g:4 * g + 4, 1:DH + 1], vp3[:, :, :],
                    bv3[:, :, :], ADD)

            # earliest-first-exp: k0 + q0 projections only, then the
            # first j-batch starts; q1..q3 interleave into it below.
            emit_kproj(0, st3[0][:, 0:512])
            emit_qproj(0, st3[1][:, 0:512])

            # ---- main loop ----------------------------------------
            from collections import deque
            pend_pv = deque()

            def emit_qk_half(stx, e, jbase, half):
                # 2 M=128 tiles (row groups 2h, 2h+1), concurrent; full
                # 128-col weight loads also enable FWL on fp16
                for b in range(2):
                    r = 2 * half + b
                    jslice = jbase + 128 * r
                    kt = kzc[jslice // 512]
                    off = jslice % 512
                    nc.tensor.matmul(
                        stx[0:128, 512 * b:512 * (b + 1)],
                        lhsT=kt[32 * r:32 * r + 32, off:off + 128],
                        rhs=qzc[e][32 * r:32 * r + 32, :],
                        start=True, stop=True,
                        tile_position=(32 * r, 0))

            exp_ctr = [0]

            def emit_exp(stx):
                pt = ptp.tile([128, 1024], bf16, tag="pt")
                i = exp_ctr[0]
                exp_ctr[0] += 1
                dve = ((i + 1) * DVE_NUM) // DVE_DEN > (i * DVE_NUM) // DVE_DEN
                if dve:
                    nc.vector.tensor_scalar(pt[:].bitcast(i16), stx[:],
                                            SCALE16, MAGIC16, MULT, ADD)
                else:
                    nc.scalar.activation(pt[:], stx[:], EXP,
                                         bias=zerob[:], scale=LN2)
                return pt

            def make_pv(pts, jbase):
                # pts[s] = (pt_half0, pt_half1) for strip/eighth s.
                def emit_m(m):
                    def emit():
                        jc = jbase // 128 + m
                        va = vaugT[:, 32 * jc:32 * jc + 32]
                        for s in range(4):
                            nc.tensor.matmul(
                                accb[32 * s:32 * s + 32, :],
                                lhsT=va,
                                rhs=pts[s][m // 2][:, 512 * (m % 2):
                                                   512 * (m % 2) + 512],
                                start=(jc == 0), stop=(jc == NCHUNK - 1),
                                tile_position=(0, 32 * s),
                                skip_group_check=True)
                    return emit
                return [emit_m(m) for m in range(4)]

            def make_fin(q):
                def fin():
                    # unnormalized output + denominators to HBM; the
                    # divide happens on host
                    osb = osp.tile([128, 512], bf16, tag="osb")
                    nc.vector.tensor_copy(osb[:], accb[:])
                    nc.sync.dma_start(o_d.ap()[:, 512 * q:512 * (q + 1)],
                                      osb[:])
                return fin

            # fully continuous pipeline over all 128 tiles: exp(T-3)
            # always pairs with qk(T), across j-batch and q boundaries,
            # so the leftover-exp clump (a ~2us ScalarE hiccup per
            # boundary) never forms.  `pending` holds the 3 in-flight
            # tiles; pts_map collects a j-batch's 8 pt tiles and
            # appends its pv rounds when complete.
            from collections import deque as _dq
            pending = _dq()  # (jbase_q, e, h, psum_tile)
            pts_map = {}
            ndone = {}

            def flush_one():
                jq, pe, ph, ptile = pending.popleft()
                q_, jbase_ = jq
                pts_map[jq][pe - 4 * q_][ph] = emit_exp(ptile)
                ndone[jq] += 1
                if ndone[jq] == 8:
                    ems = make_pv(pts_map.pop(jq), jbase_)
                    fin = make_fin(q_) if jbase_ == JB[7] else None
                    for k, em in enumerate(ems):
                        pend_pv.append((em, fin if k == 3 else None))

            for g in range(128):
                q, t = divmod(g, 64)
                jbi, k = divmod(t, 8)
                jbase = JB[jbi]
                e, h = 4 * q + k // 2, k % 2
                jq = (q, jbase)
                if jq not in pts_map:
                    pts_map[jq] = [[None, None] for _ in range(4)]
                    ndone[jq] = 0
                if len(pending) == 3:
                    flush_one()
                if g == 1:
                    emit_qproj(1, st3[2][:, 0:512])
                elif g == 3:
                    emit_qproj(2, spare[:, 0:512])
                elif g == 5:
                    emit_qproj(3, accb[:, 0:512])
                stile = st3[g % 3]
                emit_qk_half(stile, e, jbase, h)
                pending.append((jq, e, h, stile))
                if g % 2 == 1 and len(pend_pv) > 4:
                    emit, fin = pend_pv.popleft()
                    emit()
                    if fin is not None:
                        fin()
                if q == 0 and k == 7:
                    emit_vgroup(jbi)
                if g == 7:
                    # remaining projections (k1..k7, q4..q7) through the
                    # accb/spare banks (free until pv(jbase0) pops);
                    # their consumers fit the DMA-bound first block.
                    slots = [accb[:, 0:512], spare[:, 0:512]]
                    si = 0
                    for i in range(1, 8):
                        emit_kproj(i, slots[si % 2]); si += 1
                    for i in range(4, 8):
                        emit_qproj(i, slots[si % 2]); si += 1
            while pending:
                flush_one()
            while pend_pv:
                emit, fin = pend_pv.popleft()
                emit()
                if fin is not None:
                    fin()

    nc.compile()
    return nc


def _get_program():
    global _compiled
    if _compiled is None:
        _compiled = _build_program()
    return _compiled


def _to_bf16(x):
    import ml_dtypes
    return np.ascontiguousarray(
        np.asarray(x, np.float32)).astype(ml_dtypes.bfloat16)


def _prepare_core_inputs(x, w_qkv, b_qkv, emb_d, emb_h, emb_w):
    x2 = np.ascontiguousarray(
        np.asarray(x, np.float32).reshape(C, N)).astype(np.float16)
    w_qkv = np.asarray(w_qkv, np.float32)
    b_qkv = np.asarray(b_qkv, np.float32)
    qs = (DH ** -0.5) * LOG2E
    emb = (np.asarray(emb_d, np.float32)
           + np.asarray(emb_h, np.float32)
           + np.asarray(emb_w, np.float32)).reshape(DH, N)
    in_maps = []
    for h in range(NHEADS):
        qc = slice(h * DH, (h + 1) * DH)
        kc = slice(DV + h * DH, DV + (h + 1) * DH)
        vc = slice(2 * DV + h * DH, 2 * DV + (h + 1) * DH)
        wq4 = np.zeros((C, 128), np.float32)
        wk4 = np.zeros((C, 128), np.float32)
        bq4 = np.zeros((128, 1), np.float32)
        bk4 = np.zeros((128, N), np.float32)
        for r in range(4):
            wq4[:, 32 * r:32 * r + DH] = w_qkv[:, qc] * qs
            wk4[:, 32 * r:32 * r + DH] = w_qkv[:, kc]
            bq4[32 * r:32 * r + DH, 0] = b_qkv[qc] * qs
            bk4[32 * r:32 * r + DH, :] = b_qkv[kc][:, None] + emb
        bvp = np.tile(b_qkv[vc][None, :], (128, 32))
        head = np.concatenate(
            [x2[:, 0:2048], wq4.astype(np.float16),
             wk4.astype(np.float16),
             bq4.astype(np.float32).view(np.float16),
             np.zeros((128, 126), np.float16)], axis=1)
        in_maps.append({
            "head": np.ascontiguousarray(head),
            "x": x2,
            "wv": np.ascontiguousarray(w_qkv[:, vc]).astype(np.float16),
            "bk4a": _to_bf16(np.ascontiguousarray(bk4[:, 0:1024])),
            "bk4b": _to_bf16(np.ascontiguousarray(bk4[0:32, 1024:4096])),
            "bvp": _to_bf16(bvp),
        })
    return in_maps


def kernel(x, w_qkv, b_qkv, emb_d, emb_h, emb_w):
    from concourse.bass_utils import run_bass_kernel_spmd

    nc = _get_program()
    in_maps = _prepare_core_inputs(x, w_qkv, b_qkv, emb_d, emb_h, emb_w)
    res = run_bass_kernel_spmd(nc, in_maps, list(range(NHEADS)))
    out = np.empty((DV, N), np.float32)
    for h in range(NHEADS):
        r = np.asarray(res.results[h]["out"], np.float32)  # [128, 1024]
        for q in range(2):
            blk = r[:, 512 * q:512 * (q + 1)]
            for s in range(4):
                e = 4 * q + s
                denom = blk[32 * s, :]
                out[h * DH:(h + 1) * DH, e * IW:(e + 1) * IW] = (
                    blk[32 * s + 1:32 * s + 17, :] / denom)
    return out.reshape(1, DV, 16, 16, 16)
